# revision 22
# baseline (speedup 1.0000x reference)
"""Trainium2 Bass kernel for nn_CCALoss (CLIP + masked concept BCE + Jaccard-KL loss).

Contract: kernel(**inputs) takes the FULL unsharded inputs and returns the FULL
(scalar) output. Batch rows are sharded across 8 NeuronCores; each core computes
per-row partial sums; the host does the O(B) finalization in fp64.

v3 design (ScalarE-bound): the four exp streams (img, txt, csim, Jaccard sim)
are the irreducible work — 16 x [128, 4096] ACTIVATEs per core at 1 elem/cyc
(~60us). Everything else is kept strictly under that:
  - host pre-casts img/txt to fp8, csim to bf16, pre-transposes u/v concept
    indicators into matmul-ready fp8 layout, precomputes row sums rs (exact in
    fp16; also prescaled by 1/TEMP).
  - all input DMAs are issued up front (bufs=4, no buffer-reuse semaphores),
    split across the sync/gpsimd/scalar queues.
  - TensorE builds union = rs_i + rs_j - inter directly in PSUM (fp8 DoubleRow
    GEMMs + K=2 fp16 fold matmuls).
  - DVE per strip: reciprocal, s~ = (rs_i+rs_j)/T * q, d = s~ - csim (in place),
    and ONE merged accumulation  esc = sum e*(s - csim)  [since the KL row term
    only needs Zs, Zc and sum t*(s - csim)].
  - ScalarE runs only activations; e-ACTs are delayed one strip so they never
    stall on the matmul->reciprocal chain.
"""

import numpy as np
import ml_dtypes

import concourse.bacc as bacc
import concourse.bass as bass
import concourse.tile as tile
from concourse import mybir
from concourse import dve_ops
from concourse.bass_utils import run_bass_kernel_spmd
from concourse.dve_spec import AluOp, Bin, C0, C1, Spec, Src0, Src1, lower
from concourse.dve_uop import DveOpSpec


def _register_recip_mul():
    """Custom DVE op: out = approx_recip(in0) * in1 (seed + 1 Newton step).

    Replaces the reciprocal_approx_fast + scalar_tensor_tensor pair (two 1x
    DVE passes) with a single pass. ~0.17% max rel error on the union range.
    Registered via the documented dve_ops extension point (OPS append).
    """
    name = "RECIP_MUL_ANT"
    for op in dve_ops.OPS:
        if op.name == name:
            return op

    def _ref(in0, in1, c0, c1, c2):
        notx = (~in0.view(np.int32)).view(np.float32)
        y0 = notx * c0
        y1 = (y0 * (c1 - in0.astype(np.float32) * y0)).astype(np.float32)
        return (y1 * in1).astype(np.float32)

    _notx = Bin(AluOp.BITWISE_NOT, Src0, Src0)
    _y0 = _notx * C0
    _y1 = _y0 * (C1 - Src0 * _y0)
    op = dve_ops.DveOp(name, Spec(body=_y1 * Src1, reference=_ref),
                       subdim=False, uops_sha={})
    dve_ops.OPS.append(op)
    dve_ops.CUSTOM_DVE_SPECS[name] = op.spec
    dve_ops._SUB_OPCODE_FOR_NAME[name] = (
        dve_ops._CUSTOM_DVE_ROW_BASE + len(dve_ops.OPS) - 1)
    for ver in ("v3", "v4"):
        spec = DveOpSpec(name=name, opcode=dve_ops.get_dve_sub_opcode(name),
                         uops=lower(op.spec, ver=ver), rd1_en=True)
        op.uops_sha[ver] = spec.sha(ver)
    return op


RECIP_MUL = _register_recip_mul()
RM_C = dve_ops.RECIP_APPROX_FAST_CONSTS  # optimal for seed + 1 NR too

B = 4096
C = 512
NCORES = 8
R = B // NCORES  # 512 rows per core
RT = R // 128    # 4 row tiles per core
TEMP = 0.07
CONCEPT_WEIGHT = 0.5
CONCEPT_SIM_WEIGHT = 0.3

FP32 = mybir.dt.float32
FP16 = mybir.dt.float16
BF16 = mybir.dt.bfloat16
FP8 = mybir.dt.float8e4
AX = mybir.AxisListType
ALU = mybir.AluOpType
ACTF = mybir.ActivationFunctionType
DR = mybir.MatmulPerfMode.DoubleRow

NP_FP8 = ml_dtypes.float8_e4m3
NP_BF16 = ml_dtypes.bfloat16

# stat rows in the [6, RT, 128] per-core output tensor
O_ZIMG, O_ZTXT, O_ZC, O_ZS, O_ESC, O_B = range(6)


def build_nc():
    nc = bacc.Bacc("TRN2", target_bir_lowering=False, debug=False)

    img = nc.dram_tensor("img", [R, B], FP8, kind="ExternalInput")
    txt = nc.dram_tensor("txt", [R, B], FP8, kind="ExternalInput")
    csim = nc.dram_tensor("csim", [R, B], BF16, kind="ExternalInput")
    u8 = nc.dram_tensor("u8", [128, 8, B], FP8, kind="ExternalInput")
    nuv = nc.dram_tensor("nuv", [128, 8, R], FP8, kind="ExternalInput")
    rsb = nc.dram_tensor("rsb", [B], FP16, kind="ExternalInput")  # rs/T
    rst = nc.dram_tensor("rst", [2, B], FP16, kind="ExternalInput")
    lst = nc.dram_tensor("lst", [2, R], FP16, kind="ExternalInput")
    rsloc = nc.dram_tensor("rsloc", [128, RT], FP32, kind="ExternalInput")
    clsp = nc.dram_tensor("clsp", [128, RT * C], BF16, kind="ExternalInput")
    clv = nc.dram_tensor("clv", [128, RT * C], BF16, kind="ExternalInput")
    out = nc.dram_tensor("out", [6, RT, 128], FP32, kind="ExternalOutput")

    with tile.TileContext(nc) as tc:
        _build(nc, tc, img, txt, csim, u8, nuv, rsb, rst, lst, rsloc,
               clsp, clv, out)
    nc.compile()
    return nc


def _build(nc, tc, img, txt, csim, u8, nuv, rsb, rst, lst, rsloc, clsp, clv,
           out):
    from contextlib import ExitStack

    inv_t = float(1.0 / TEMP)

    ctx = ExitStack()
    with ctx:
        singles = ctx.enter_context(tc.tile_pool(name="singles", bufs=1))
        io = ctx.enter_context(tc.tile_pool(name="io", bufs=4))
        wrk = ctx.enter_context(tc.tile_pool(name="wrk", bufs=2))
        scrp = ctx.enter_context(tc.tile_pool(name="scr", bufs=1))
        psp = ctx.enter_context(tc.tile_pool(name="ps", bufs=2, space="PSUM"))

        # ---------------- upfront DMAs ----------------
        # scalar queue: u/v concept matrices (ScalarE is idle at the start;
        # the transfers stream while the first img/txt exps run)
        U8c = []
        for cp in range(4):
            t = singles.tile([128, 2, B], FP8, name=f"u8c{cp}")
            eng = [nc.scalar, nc.scalar, nc.sync, nc.gpsimd][cp]
            eng.dma_start(out=t, in_=u8.ap()[:, 2 * cp:2 * cp + 2, :])
            U8c.append(t)
        nUVs = singles.tile([128, 8, R], FP8)
        nc.scalar.dma_start(out=nUVs, in_=nuv.ap())

        rstS = singles.tile([2, B], FP16)
        nc.sync.dma_start(out=rstS, in_=rst.ap())
        lstS = singles.tile([2, R], FP16)
        nc.sync.dma_start(out=lstS, in_=lst.ap())
        rslocS = singles.tile([128, RT], FP32)
        nc.sync.dma_start(out=rslocS, in_=rsloc.ap())
        # rs_j/T broadcast across partitions (stride-0 partition DMA)
        rsbc = singles.tile([128, B], FP16)
        nc.sync.dma_start(
            out=rsbc,
            in_=bass.AP(tensor=rsb.ap().tensor, offset=0, ap=[[0, 128], [1, B]]))

        strip_tiles = {}
        for ic in range(RT):
            i0 = ic * 128
            imt = io.tile([128, B], FP8, tag="img", name=f"img{ic}")
            nc.sync.dma_start(out=imt, in_=img[i0:i0 + 128, :])
            txtt = io.tile([128, B], FP8, tag="txt", name=f"txt{ic}")
            nc.gpsimd.dma_start(out=txtt, in_=txt[i0:i0 + 128, :])
            cst = io.tile([128, B], BF16, tag="cs", name=f"cs{ic}")
            nc.gpsimd.dma_start(out=cst, in_=csim[i0:i0 + 128, :])
            strip_tiles[ic] = (imt, txtt, cst)

        clspS = singles.tile([128, RT * C], BF16)
        nc.sync.dma_start(out=clspS, in_=clsp.ap())
        clvS = singles.tile([128, RT * C], BF16)
        nc.sync.dma_start(out=clvS, in_=clv.ap())

        # ---------------- persistent tiles ----------------
        partsA = singles.tile([128, 6, RT], FP32)
        nc.vector.memset(partsA, 0.0)
        minvt_col = singles.tile([128, 1], FP32)
        nc.vector.memset(minvt_col, -inv_t)
        one_col = singles.tile([128, 1], FP32)
        nc.vector.memset(one_col, 1.0)
        junk8 = singles.tile([128, B], FP8)     # dummy ACT output
        junkv = singles.tile([128, RT * C], BF16)
        junkv2 = singles.tile([128, RT * C], BF16)

        # ---------------- main loop over row tiles ----------------
        def emit_e(ic, st, cstp):
            # e = exp((sp1 - 1)/T) = exp(sim/T), fused row-sum -> Zs
            e = wrk.tile([128, B], BF16, tag="e", name=f"e{ic}")
            nc.scalar.activation(e, st, ACTF.Exp, bias=minvt_col,
                                 scale=inv_t,
                                 accum_out=partsA[:, O_ZS, ic:ic + 1])
            # d = sp1 - T*csim (in place over sp1), then
            # esc_raw = sum e*(d - 1) = T * sum e*(s - csim)
            nc.vector.tensor_tensor(st, st, cstp, ALU.subtract)
            scr = scrp.tile([128, B], BF16, tag="esc")
            nc.vector.scalar_tensor_tensor(
                scr, st, -1.0, e, ALU.add, ALU.mult,
                accum_out=partsA[:, O_ESC, ic:ic + 1])

        prev = None
        for ic in range(RT):
            i0 = ic * 128
            imt, txtt, cst = strip_tiles.pop(ic)

            # independent ACT work first (csim arrives pre-scaled by T, so
            # Zc uses scale=1/T to recover exp(csim))
            nc.scalar.activation(junk8, imt, ACTF.Exp,
                                 accum_out=partsA[:, O_ZIMG, ic:ic + 1])
            nc.scalar.activation(junk8, txtt, ACTF.Exp,
                                 accum_out=partsA[:, O_ZTXT, ic:ic + 1])
            nc.scalar.activation(junk8, cst, ACTF.Exp, scale=inv_t,
                                 accum_out=partsA[:, O_ZC, ic:ic + 1])

            # rs_i + rs_j (fp16, exact)
            rssum = wrk.tile([128, B], FP16, tag="rssum", name=f"rss{ic}")
            nc.vector.tensor_scalar(rssum, rsbc, rslocS[:, ic:ic + 1], None,
                                    ALU.add)

            st = wrk.tile([128, B], BF16, tag="st", name=f"st{ic}")
            for h in range(2):
                j0 = h * 2048
                ps = psp.tile([128, 2048], FP32, tag="ps", name=f"ps{ic}_{h}")
                # union = -0.5*(u.uT + v.vT) + rs_i + rs_j, accumulated in PSUM
                for cp in range(4):
                    for jb in range(4):
                        nc.tensor.matmul(
                            ps[:, jb * 512:(jb + 1) * 512],
                            nUVs[:, 2 * cp:2 * cp + 2, i0:i0 + 128],
                            U8c[cp][:, :, j0 + jb * 512:j0 + (jb + 1) * 512],
                            start=(cp == 0), stop=False, perf_mode=DR)
                for jb in range(4):
                    nc.tensor.matmul(
                        ps[:, jb * 512:(jb + 1) * 512],
                        lstS[:, i0:i0 + 128],
                        rstS[:, j0 + jb * 512:j0 + (jb + 1) * 512],
                        start=False, stop=True)
                # sp1 = (rs_i + rs_j)/union = sim + 1, in one fused DVE pass
                nc.vector._custom_dve(
                    RECIP_MUL, out=st[:, j0:j0 + 2048], in0=ps,
                    in1=rssum[:, j0:j0 + 2048],
                    s0=RM_C["s0"], s1=RM_C["s1"], imm2=0.0)

            if prev is not None:
                emit_e(*prev)
            prev = (ic, st, cst)
        emit_e(*prev)

        # ---------------- BCE tail (pinned late) ----------------
        with tc.high_priority(offset=-(10 ** 6)):
            # b1 = sum softplus(clog_masked) = sum ln(exp(clog_masked) + 1)
            nc.scalar.activation(clspS, clspS, ACTF.Exp)
            nc.scalar.activation(junkv, clspS, ACTF.Ln, bias=one_col,
                                 accum_out=partsA[:, O_B, 0:1])
        # b2 = sum clog*target (host pre-masked)
        nc.vector.tensor_scalar(junkv2, clvS, 0.0, None, ALU.add, ALU.add,
                                accum_out=partsA[:, O_B, 1:2])

        nc.gpsimd.dma_start(out=out.ap().rearrange("r t p -> p r t"),
                            in_=partsA)


_NC_CACHE = None
LAST_RESULT = None


def _get_nc():
    global _NC_CACHE
    if _NC_CACHE is None:
        _NC_CACHE = build_nc()
    return _NC_CACHE


def kernel(logits_per_image, logits_per_text, concepts_logits,
           concept_image_similarity, medical_concepts):
    img = np.asarray(logits_per_image, dtype=np.float32)
    txt = np.asarray(logits_per_text, dtype=np.float32)
    csim = np.asarray(concept_image_similarity, dtype=np.float32)
    clog = np.asarray(concepts_logits, dtype=np.float32)
    mc = np.asarray(medical_concepts)

    img8 = np.ascontiguousarray(img.astype(NP_FP8))
    txt8 = np.ascontiguousarray(txt.astype(NP_FP8))
    # csim pre-scaled by T so d = sp1 - T*csim stays in bf16's sweet spot
    cs16 = np.ascontiguousarray((csim * TEMP).astype(NP_BF16))

    u = (mc != 0)
    v = (mc == 1)
    mask = (mc != -1)
    rs = 0.5 * (u.sum(axis=1, dtype=np.float64)
                + v.sum(axis=1, dtype=np.float64))  # exact halves <= 512

    # matmul-ready transposed layout: U8_full[p, cc, j] = u.T/v.T chunks
    uT = u.T.astype(NP_FP8).reshape(4, 128, B)
    vT = v.T.astype(NP_FP8).reshape(4, 128, B)
    U8_full = np.ascontiguousarray(
        np.concatenate([uT, vT], axis=0).transpose(1, 0, 2))  # [128, 8, B]
    nUV_full = (-0.5 * np.concatenate([uT, vT], axis=0).astype(np.float32))
    nUV_full = nUV_full.transpose(1, 0, 2).astype(NP_FP8)  # [128, 8, B]

    rs16 = rs.astype(np.float16)          # exact (halves <= 512)
    rst_h = np.ones((2, B), dtype=np.float16)
    rst_h[0] = rs16
    rst_h = np.ascontiguousarray(rst_h)

    clog_sp = np.where(mask, clog, -30.0).astype(NP_BF16)
    clog_v = np.where(v, clog, 0.0).astype(NP_BF16)

    nc = _get_nc()
    in_maps = []
    for c in range(NCORES):
        g0 = c * R
        lst_h = np.ones((2, R), dtype=np.float16)
        lst_h[1] = rs16[g0:g0 + R]
        rsloc_h = np.ascontiguousarray(
            rs[g0:g0 + R].astype(np.float32).reshape(RT, 128).T)
        in_maps.append({
            "img": img8[g0:g0 + R],
            "txt": txt8[g0:g0 + R],
            "csim": cs16[g0:g0 + R],
            "u8": U8_full,
            "nuv": np.ascontiguousarray(nUV_full[:, :, g0:g0 + R]),
            "rsb": rs16,
            "rst": rst_h,
            "lst": lst_h,
            "rsloc": rsloc_h,
            "clsp": np.ascontiguousarray(
                clog_sp[g0:g0 + R].reshape(RT, 128, C).transpose(1, 0, 2)
                .reshape(128, RT * C)),
            "clv": np.ascontiguousarray(
                clog_v[g0:g0 + R].reshape(RT, 128, C).transpose(1, 0, 2)
                .reshape(128, RT * C)),
        })
    res = run_bass_kernel_spmd(nc, in_maps, list(range(NCORES)))
    global LAST_RESULT
    LAST_RESULT = res

    outs = [r["out"].astype(np.float64) for r in res.results]  # [6, RT, 128]
    rows = {k: np.concatenate([o[k].reshape(R) for o in outs])
            for k in (O_ZIMG, O_ZTXT, O_ZC, O_ZS, O_ESC)}
    b1 = sum(o[O_B, 0, :].sum() for o in outs)
    b2 = sum(o[O_B, 1, :].sum() for o in outs)

    diag_i = np.diagonal(img).astype(np.float64)
    diag_t = np.diagonal(txt).astype(np.float64)
    clip_loss = 0.5 * (np.mean(np.log(rows[O_ZIMG]) - diag_i)
                       + np.mean(np.log(rows[O_ZTXT]) - diag_t))

    ms = float(mask.sum())
    concept_loss = (b1 - b2) / (ms + 1e-8)

    # kl_i = (1/Zs)*sum_j e*(s - csim) - log Zs + log Zc;  esc_raw = T*sum(...)
    zs, esc, zc = rows[O_ZS], rows[O_ESC], rows[O_ZC]
    kl = np.mean((esc / TEMP) / zs - np.log(zs) + np.log(zc))

    total = clip_loss + CONCEPT_WEIGHT * concept_loss + CONCEPT_SIM_WEIGHT * kl
    return np.float32(total)
